# revision 28
# baseline (speedup 1.0000x reference)
"""MoE (top-2 of 8 experts) Trainium2 kernel.

Strategy: expert-parallel across the 8 NeuronCores. The host computes
LayerNorm + gating (0.1% of FLOPs) in numpy, routes each token's top-2
experts, and hands core `e` the tokens routed to expert `e` in a
transposed [D, C] layout. Each core runs the FFN for its expert in bf16
(fp32 PSUM accumulation):

    yT = g * (W2.T @ gelu(W1.T @ xgT + b1) + b2)

Keeping activations feature-major means both matmuls consume the weights
in their natural layout: no on-chip transposes. The host scatter-adds the
two expert contributions per token back together and adds the residual.
"""

import os
import sys
import numpy as np
import ml_dtypes
from contextlib import ExitStack

if "/opt/trn_rl_repo" not in sys.path:
    sys.path.insert(0, "/opt/trn_rl_repo")

import concourse.bass as bass
import concourse.bacc as bacc
import concourse.tile as tile
from concourse.tile import add_dep_helper
from concourse import mybir
from concourse import bass_utils

P = 128          # partitions
EPS = 1e-5
N_CORES = 8

_prog_cache = {}


def _chunks(C):
    """Split C tokens into the fewest ≤512 near-equal chunks."""
    k = -(-C // 512)
    base = C // k
    sizes = [base + (1 if i < C - base * k else 0) for i in range(k)]
    offs = np.concatenate([[0], np.cumsum(sizes)[:-1]]).astype(int)
    return list(zip(offs.tolist(), sizes))


def _build_program(C, D, H, act="gelu"):
    """One SPMD program, identical on all 8 cores (per-core data differs)."""
    KD = D // P      # D chunks (8)
    KH = H // P      # H chunks (32)
    chunks = _chunks(C)
    NMAX = max(n for _, n in chunks)

    nc = bacc.Bacc("TRN2", target_bir_lowering=False, debug=False,
                   num_devices=N_CORES)
    bf16 = mybir.dt.bfloat16
    f32 = mybir.dt.float32

    xgT = nc.dram_tensor("xgT", [D, C], bf16, kind="ExternalInput")
    w1 = nc.dram_tensor("w1", [D, H], bf16, kind="ExternalInput")
    w2 = nc.dram_tensor("w2", [H, D], bf16, kind="ExternalInput")
    b1 = nc.dram_tensor("b1", [H], f32, kind="ExternalInput")
    b2 = nc.dram_tensor("b2", [D], f32, kind="ExternalInput")
    gb = nc.dram_tensor("gb", [P, C], f32, kind="ExternalInput")
    yT = nc.dram_tensor("yT", [D, C], f32, kind="ExternalOutput")

    gelu = (mybir.ActivationFunctionType.Gelu if act == "gelu"
            else mybir.ActivationFunctionType.Identity)
    ident = mybir.ActivationFunctionType.Identity

    DC1 = KD // 2  # output chunks accumulated in-line with phase A

    with tile.TileContext(nc) as tc, ExitStack() as ctx:
        wpool = ctx.enter_context(tc.tile_pool(name="weights", bufs=1))
        xpool = ctx.enter_context(tc.tile_pool(name="xg", bufs=2))
        gpool = ctx.enter_context(tc.tile_pool(name="g", bufs=2))
        hpool = ctx.enter_context(tc.tile_pool(name="h", bufs=1))
        ypool = ctx.enter_context(tc.tile_pool(name="y", bufs=3))
        psh_pool = ctx.enter_context(
            tc.tile_pool(name="psh", bufs=2, space="PSUM"))
        psy_pool = ctx.enter_context(
            tc.tile_pool(name="psy", bufs=6, space="PSUM"))

        # Weights resident in SBUF, natural layout, partition = contraction
        # dim. DMAs sliced ~1MB in consumption order so the PE can start as
        # soon as the first slices land instead of waiting for whole tensors.
        w1s = wpool.tile([P, KD, H], bf16)
        w2s = wpool.tile([P, KH, D], bf16)
        b1s = wpool.tile([P, KH], f32)
        b2s = wpool.tile([P, KD], f32)

        xgT_r = xgT.ap().rearrange("(kc p) t -> p kc t", p=P)
        yT_r = yT.ap().rearrange("(dc p) t -> p dc t", p=P)
        w1_r = w1.ap().rearrange("(kc p) h -> p kc h", p=P)
        w2_r = w2.ap().rearrange("(kc p) d -> p kc d", p=P)

        o0, n0 = chunks[0]
        xgs0 = xpool.tile([P, KD, NMAX], bf16, tag="xgs")
        for kc in range(KD):
            nc.sync.dma_start(xgs0[:, kc, 0:n0], xgT_r[:, kc, o0:o0 + n0])
        nc.sync.dma_start(b1s[:], b1.ap().rearrange("(c p) -> p c", p=P))
        # w1 H-blocks sized fine at the front so hc=0 can start early
        w1_blocks = [(0, 128), (128, 128), (256, 256)] + \
            [(off, 512) for off in range(512, H, 512)]
        WB = 4            # w2 hc rows per DMA slice (1 MB)
        w2_blocks = list(range(H // P // WB))
        # upfront: only what the first ~8 H-chunks of chunk 0 need (~4MB),
        # so the critical first blocks aren't racing 17MB of weight stream.
        # The rest is gated on the first gelu (see below) and hides behind
        # ~80us of chunk-0 compute.
        tier1, tier2 = [], []
        for off, sz in w1_blocks:
            if off == 0:
                nc.sync.dma_start(w1s[:, :, off:off + sz],
                                  w1_r[:, :, off:off + sz])
            elif off < 1024:
                tier1.append((w1s[:, :, off:off + sz],
                              w1_r[:, :, off:off + sz]))
            else:
                tier2.append((w1s[:, :, off:off + sz],
                              w1_r[:, :, off:off + sz]))
        for blk in w2_blocks:
            (tier1 if blk < 2 else tier2).append(
                (w2s[:, blk * WB:(blk + 1) * WB, :],
                 w2_r[:, blk * WB:(blk + 1) * WB, :]))
        tier1.append((b2s[:], b2.ap().rearrange("(c p) -> p c", p=P)))

        for j, (o, n) in enumerate(chunks):
            if j == 0:
                xgs = xgs0
            else:
                xgs = xpool.tile([P, KD, NMAX], bf16, tag="xgs")
                nc.sync.dma_start(xgs[:, :, 0:n], xgT_r[:, :, o:o + n])
            gbs = gpool.tile([P, NMAX], f32, tag="gbs")
            nc.sync.dma_start(gbs[:, 0:n], gb.ap()[:, o:o + n])
            hts = hpool.tile([P, KH, NMAX], bf16)
            # first DC1 output accumulators ride one hc behind phase A so W2
            # is consumed incrementally as its DMA slices arrive
            psy1 = [psy_pool.tile([P, NMAX], f32, tag="psy",
                                  name=f"psy1_{j}_{i}") for i in range(DC1)]

            def b_mms(h0):
                for dc in range(DC1):
                    nc.tensor.matmul(
                        psy1[dc][:, 0:n],
                        w2s[:, h0, dc * P:(dc + 1) * P],
                        hts[:, h0, 0:n],
                        start=(h0 == 0), stop=(h0 == KH - 1))

            for hc in range(KH):
                psh = psh_pool.tile([P, NMAX], f32, tag="psh")
                for kc in range(KD):
                    mm = nc.tensor.matmul(
                        psh[:, 0:n],
                        w1s[:, kc, hc * P:(hc + 1) * P],
                        xgs[:, kc, 0:n],
                        start=(kc == 0), stop=(kc == KD - 1))
                    if j == 0 and hc == 0 and kc == 0:
                        for dst, srcap in tier1:
                            di = nc.sync.dma_start(dst, srcap).ins
                            add_dep_helper(di, mm.ins, sync=True,
                                           reason="tier1 weights after first mm")
                        tier1 = []
                g_inst = nc.scalar.activation(hts[:, hc, 0:n], psh[:, 0:n],
                                              gelu, bias=b1s[:, hc:hc + 1])
                if j == 0 and hc == 0:
                    for dst, srcap in tier2:
                        di = nc.sync.dma_start(dst, srcap).ins
                        add_dep_helper(di, g_inst.ins, sync=True,
                                       reason="defer bulk weights")
                    tier2 = []
                if hc >= 1:
                    b_mms(hc - 1)
            b_mms(KH - 1)

            def finalize(dc, psy_t):
                ys = ypool.tile([P, NMAX], f32, tag="ys")
                nc.scalar.activation(ys[:, 0:n], psy_t[:, 0:n], ident,
                                     bias=b2s[:, dc:dc + 1])
                nc.vector.tensor_mul(ys[:, 0:n], ys[:, 0:n], gbs[:, 0:n])
                nc.sync.dma_start(yT_r[:, dc, o:o + n], ys[:, 0:n])

            for dc in range(DC1):
                finalize(dc, psy1[dc])

            # remaining output chunks: hts fully staged, plain accumulation
            for dc in range(DC1, KD):
                psy = psy_pool.tile([P, NMAX], f32, tag="psy")
                for hc in range(KH):
                    nc.tensor.matmul(
                        psy[:, 0:n],
                        w2s[:, hc, dc * P:(dc + 1) * P],
                        hts[:, hc, 0:n],
                        start=(hc == 0), stop=(hc == KH - 1))
                finalize(dc, psy)

    nc.compile()
    return nc


def _route(xf, gamma, beta, gate_w):
    """Host LayerNorm + top-2 gating in float64 (routing decisions verified
    stable across fp32/fp64/jax backends for this problem's margins)."""
    T = xf.shape[0]
    xd = xf.astype(np.float64)
    mu = xd.mean(-1, keepdims=True)
    var = ((xd - mu) ** 2).mean(-1, keepdims=True)
    xn = (xd - mu) / np.sqrt(var + EPS) * gamma.astype(np.float64) \
        + beta.astype(np.float64)
    logits = xn @ gate_w.T.astype(np.float64)
    ar = np.arange(T)
    i1 = logits.argmax(-1)
    v1 = logits[ar, i1]
    l2 = logits.copy()
    l2[ar, i1] = -np.inf
    i2 = l2.argmax(-1)
    v2 = logits[ar, i2]
    # softmax over the two selected logits (v1 >= v2)
    e2 = np.exp(v2 - v1)
    g1 = 1.0 / (1.0 + e2)
    g2 = e2 / (1.0 + e2)
    return xn, i1, i2, g1, g2


def kernel(**inputs):
    x = np.asarray(inputs["x"], np.float32)
    gamma = np.asarray(inputs["gamma"], np.float32)
    beta = np.asarray(inputs["beta"], np.float32)
    gate_w = np.asarray(inputs["gate_w"], np.float32)
    W1 = np.asarray(inputs["W1"], np.float32)
    b1 = np.asarray(inputs["b1"], np.float32)
    W2 = np.asarray(inputs["W2"], np.float32)
    b2 = np.asarray(inputs["b2"], np.float32)

    B, L, D = x.shape
    E, _, H = W1.shape
    T = B * L
    xf = x.reshape(T, D)

    xn, i1, i2, g1, g2 = _route(xf, gamma, beta, gate_w)

    # balance loss from the dense gates (host, fp64 -> fp32)
    load = np.zeros(E, np.float64)
    np.add.at(load, i1, g1)
    np.add.at(load, i2, g2)
    load /= T
    bal = np.float32(((load - 1.0 / E) ** 2).mean())

    # per-expert token lists + each token's slot position
    counts = np.bincount(np.concatenate([i1, i2]), minlength=E)
    C = int(counts.max())

    posA = np.empty(T, np.int64)
    posB = np.empty(T, np.int64)
    idx_list, gate_list = [], []
    for e in range(E):
        a = np.where(i1 == e)[0]
        b = np.where(i2 == e)[0]
        posA[a] = np.arange(len(a))
        posB[b] = len(a) + np.arange(len(b))
        idx_list.append(np.concatenate([a, b]))
        gate_list.append(np.concatenate([g1[a], g2[b]]))

    key = (C, D, H)
    if key not in _prog_cache:
        _prog_cache[key] = _build_program(C, D, H)
    nc = _prog_cache[key]

    in_maps = []
    for e in range(E):
        idx = idx_list[e]
        n = len(idx)
        xg = np.zeros((C, D), ml_dtypes.bfloat16)
        xg[:n] = xn[idx].astype(ml_dtypes.bfloat16)
        g = np.zeros(C, np.float32)
        g[:n] = gate_list[e]
        in_maps.append({
            "xgT": np.ascontiguousarray(xg.T),
            "w1": W1[e].astype(ml_dtypes.bfloat16),
            "w2": W2[e].astype(ml_dtypes.bfloat16),
            "b1": b1[e],
            "b2": b2[e],
            "gb": np.ascontiguousarray(np.broadcast_to(g, (P, C))),
        })

    res = bass_utils.run_bass_kernel_spmd(
        nc, in_maps, core_ids=list(range(N_CORES)),
        trace=bool(int(os.environ.get("MOE_TRACE", "0"))))
    if res.exec_time_ns is not None:
        print(f"HW exec time: {res.exec_time_ns} ns")

    # combine: out[t] = x[t] + y[slotA(t)] + y[slotB(t)]
    Y = np.stack([res.results[e]["yT"] for e in range(E)])  # (E, D, C)
    Yf = np.ascontiguousarray(Y.transpose(0, 2, 1)).reshape(E * C, D)
    out = xf + Yf[i1 * C + posA] + Yf[i2 * C + posB]
    return out.reshape(B, L, D).astype(np.float32), bal


# revision 29
# speedup vs baseline: 1.0096x; 1.0096x over previous
"""MoE (top-2 of 8 experts) Trainium2 kernel.

Strategy: expert-parallel across the 8 NeuronCores. The host computes
LayerNorm + gating (0.1% of FLOPs) in numpy, routes each token's top-2
experts, and hands core `e` the tokens routed to expert `e` in a
transposed [D, C] layout. Each core runs the FFN for its expert in bf16
(fp32 PSUM accumulation):

    yT = g * (W2.T @ gelu(W1.T @ xgT + b1) + b2)

Keeping activations feature-major means both matmuls consume the weights
in their natural layout: no on-chip transposes. The host scatter-adds the
two expert contributions per token back together and adds the residual.
"""

import os
import sys
import numpy as np
import ml_dtypes
from contextlib import ExitStack

if "/opt/trn_rl_repo" not in sys.path:
    sys.path.insert(0, "/opt/trn_rl_repo")

import concourse.bass as bass
import concourse.bacc as bacc
import concourse.tile as tile
from concourse.tile import add_dep_helper
from concourse import mybir
from concourse import bass_utils

P = 128          # partitions
EPS = 1e-5
N_CORES = 8

_prog_cache = {}


def _chunks(C):
    """Split C tokens into the fewest ≤512 near-equal chunks."""
    k = -(-C // 512)
    base = C // k
    sizes = [base + (1 if i < C - base * k else 0) for i in range(k)]
    offs = np.concatenate([[0], np.cumsum(sizes)[:-1]]).astype(int)
    return list(zip(offs.tolist(), sizes))


def _build_program(C, D, H, act="gelu"):
    """One SPMD program, identical on all 8 cores (per-core data differs)."""
    KD = D // P      # D chunks (8)
    KH = H // P      # H chunks (32)
    chunks = _chunks(C)
    NMAX = max(n for _, n in chunks)

    nc = bacc.Bacc("TRN2", target_bir_lowering=False, debug=False,
                   num_devices=N_CORES)
    bf16 = mybir.dt.bfloat16
    f32 = mybir.dt.float32

    xgT = nc.dram_tensor("xgT", [D, C], bf16, kind="ExternalInput")
    w1 = nc.dram_tensor("w1", [D, H], bf16, kind="ExternalInput")
    w2 = nc.dram_tensor("w2", [H, D], bf16, kind="ExternalInput")
    b1 = nc.dram_tensor("b1", [H], f32, kind="ExternalInput")
    b2 = nc.dram_tensor("b2", [D], f32, kind="ExternalInput")
    gb = nc.dram_tensor("gb", [P, C], f32, kind="ExternalInput")
    yT = nc.dram_tensor("yT", [D, C], f32, kind="ExternalOutput")

    gelu = (mybir.ActivationFunctionType.Gelu if act == "gelu"
            else mybir.ActivationFunctionType.Identity)
    ident = mybir.ActivationFunctionType.Identity

    DC1 = KD // 2  # output chunks accumulated in-line with phase A

    with tile.TileContext(nc) as tc, ExitStack() as ctx:
        wpool = ctx.enter_context(tc.tile_pool(name="weights", bufs=1))
        xpool = ctx.enter_context(tc.tile_pool(name="xg", bufs=2))
        gpool = ctx.enter_context(tc.tile_pool(name="g", bufs=2))
        hpool = ctx.enter_context(tc.tile_pool(name="h", bufs=1))
        ypool = ctx.enter_context(tc.tile_pool(name="y", bufs=3))
        psh_pool = ctx.enter_context(
            tc.tile_pool(name="psh", bufs=2, space="PSUM"))
        psy_pool = ctx.enter_context(
            tc.tile_pool(name="psy", bufs=6, space="PSUM"))

        # Weights resident in SBUF, natural layout, partition = contraction
        # dim. DMAs sliced ~1MB in consumption order so the PE can start as
        # soon as the first slices land instead of waiting for whole tensors.
        w1s = wpool.tile([P, KD, H], bf16)
        w2s = wpool.tile([P, KH, D], bf16)
        b1s = wpool.tile([P, KH], f32)
        b2s = wpool.tile([P, KD], f32)

        xgT_r = xgT.ap().rearrange("(kc p) t -> p kc t", p=P)
        yT_r = yT.ap().rearrange("(dc p) t -> p dc t", p=P)
        w1_r = w1.ap().rearrange("(kc p) h -> p kc h", p=P)
        w2_r = w2.ap().rearrange("(kc p) d -> p kc d", p=P)

        o0, n0 = chunks[0]
        xgs0 = xpool.tile([P, KD, NMAX], bf16, tag="xgs")
        for kc in range(KD):
            nc.sync.dma_start(xgs0[:, kc, 0:n0], xgT_r[:, kc, o0:o0 + n0])
        nc.sync.dma_start(b1s[:], b1.ap().rearrange("(c p) -> p c", p=P))
        # w1 H-blocks sized fine at the front so hc=0 can start early
        w1_blocks = [(0, 128), (128, 128), (256, 256)] + \
            [(off, 512) for off in range(512, H, 512)]
        WB = 4            # w2 hc rows per DMA slice (1 MB)
        w2_blocks = list(range(H // P // WB))
        # upfront: only what the first ~8 H-chunks of chunk 0 need (~4MB),
        # so the critical first blocks aren't racing 17MB of weight stream.
        # The rest is gated on the first gelu (see below) and hides behind
        # ~80us of chunk-0 compute.
        deferred = []
        for off, sz in w1_blocks:
            if off < 1024:
                nc.sync.dma_start(w1s[:, :, off:off + sz],
                                  w1_r[:, :, off:off + sz])
            else:
                deferred.append((w1s[:, :, off:off + sz],
                                 w1_r[:, :, off:off + sz]))
        for blk in w2_blocks:
            if blk < 2:
                nc.sync.dma_start(w2s[:, blk * WB:(blk + 1) * WB, :],
                                  w2_r[:, blk * WB:(blk + 1) * WB, :])
            else:
                deferred.append((w2s[:, blk * WB:(blk + 1) * WB, :],
                                 w2_r[:, blk * WB:(blk + 1) * WB, :]))
        nc.sync.dma_start(b2s[:], b2.ap().rearrange("(c p) -> p c", p=P))

        for j, (o, n) in enumerate(chunks):
            if j == 0:
                xgs = xgs0
            else:
                xgs = xpool.tile([P, KD, NMAX], bf16, tag="xgs")
                nc.sync.dma_start(xgs[:, :, 0:n], xgT_r[:, :, o:o + n])
            gbs = gpool.tile([P, NMAX], f32, tag="gbs")
            nc.sync.dma_start(gbs[:, 0:n], gb.ap()[:, o:o + n])
            hts = hpool.tile([P, KH, NMAX], bf16)
            # first DC1 output accumulators ride one hc behind phase A so W2
            # is consumed incrementally as its DMA slices arrive
            psy1 = [psy_pool.tile([P, NMAX], f32, tag="psy",
                                  name=f"psy1_{j}_{i}") for i in range(DC1)]

            def b_mms(h0):
                for dc in range(DC1):
                    nc.tensor.matmul(
                        psy1[dc][:, 0:n],
                        w2s[:, h0, dc * P:(dc + 1) * P],
                        hts[:, h0, 0:n],
                        start=(h0 == 0), stop=(h0 == KH - 1))

            for hc in range(KH):
                psh = psh_pool.tile([P, NMAX], f32, tag="psh")
                for kc in range(KD):
                    nc.tensor.matmul(
                        psh[:, 0:n],
                        w1s[:, kc, hc * P:(hc + 1) * P],
                        xgs[:, kc, 0:n],
                        start=(kc == 0), stop=(kc == KD - 1))
                g_inst = nc.scalar.activation(hts[:, hc, 0:n], psh[:, 0:n],
                                              gelu, bias=b1s[:, hc:hc + 1])
                if j == 0 and hc == 0:
                    for dst, srcap in deferred:
                        di = nc.sync.dma_start(dst, srcap).ins
                        add_dep_helper(di, g_inst.ins, sync=True,
                                       reason="defer bulk weights")
                    deferred = []
                if hc >= 1:
                    b_mms(hc - 1)
            b_mms(KH - 1)

            def finalize(dc, psy_t):
                ys = ypool.tile([P, NMAX], f32, tag="ys")
                nc.scalar.activation(ys[:, 0:n], psy_t[:, 0:n], ident,
                                     bias=b2s[:, dc:dc + 1])
                nc.vector.tensor_mul(ys[:, 0:n], ys[:, 0:n], gbs[:, 0:n])
                nc.sync.dma_start(yT_r[:, dc, o:o + n], ys[:, 0:n])

            for dc in range(DC1):
                finalize(dc, psy1[dc])

            # remaining output chunks: hts fully staged, plain accumulation
            for dc in range(DC1, KD):
                psy = psy_pool.tile([P, NMAX], f32, tag="psy")
                for hc in range(KH):
                    nc.tensor.matmul(
                        psy[:, 0:n],
                        w2s[:, hc, dc * P:(dc + 1) * P],
                        hts[:, hc, 0:n],
                        start=(hc == 0), stop=(hc == KH - 1))
                finalize(dc, psy)

    nc.compile()
    return nc


def _route(xf, gamma, beta, gate_w):
    """Host LayerNorm + top-2 gating in float64 (routing decisions verified
    stable across fp32/fp64/jax backends for this problem's margins)."""
    T = xf.shape[0]
    xd = xf.astype(np.float64)
    mu = xd.mean(-1, keepdims=True)
    var = ((xd - mu) ** 2).mean(-1, keepdims=True)
    xn = (xd - mu) / np.sqrt(var + EPS) * gamma.astype(np.float64) \
        + beta.astype(np.float64)
    logits = xn @ gate_w.T.astype(np.float64)
    ar = np.arange(T)
    i1 = logits.argmax(-1)
    v1 = logits[ar, i1]
    l2 = logits.copy()
    l2[ar, i1] = -np.inf
    i2 = l2.argmax(-1)
    v2 = logits[ar, i2]
    # softmax over the two selected logits (v1 >= v2)
    e2 = np.exp(v2 - v1)
    g1 = 1.0 / (1.0 + e2)
    g2 = e2 / (1.0 + e2)
    return xn, i1, i2, g1, g2


def kernel(**inputs):
    x = np.asarray(inputs["x"], np.float32)
    gamma = np.asarray(inputs["gamma"], np.float32)
    beta = np.asarray(inputs["beta"], np.float32)
    gate_w = np.asarray(inputs["gate_w"], np.float32)
    W1 = np.asarray(inputs["W1"], np.float32)
    b1 = np.asarray(inputs["b1"], np.float32)
    W2 = np.asarray(inputs["W2"], np.float32)
    b2 = np.asarray(inputs["b2"], np.float32)

    B, L, D = x.shape
    E, _, H = W1.shape
    T = B * L
    xf = x.reshape(T, D)

    xn, i1, i2, g1, g2 = _route(xf, gamma, beta, gate_w)

    # balance loss from the dense gates (host, fp64 -> fp32)
    load = np.zeros(E, np.float64)
    np.add.at(load, i1, g1)
    np.add.at(load, i2, g2)
    load /= T
    bal = np.float32(((load - 1.0 / E) ** 2).mean())

    # per-expert token lists + each token's slot position
    counts = np.bincount(np.concatenate([i1, i2]), minlength=E)
    C = int(counts.max())

    posA = np.empty(T, np.int64)
    posB = np.empty(T, np.int64)
    idx_list, gate_list = [], []
    for e in range(E):
        a = np.where(i1 == e)[0]
        b = np.where(i2 == e)[0]
        posA[a] = np.arange(len(a))
        posB[b] = len(a) + np.arange(len(b))
        idx_list.append(np.concatenate([a, b]))
        gate_list.append(np.concatenate([g1[a], g2[b]]))

    key = (C, D, H)
    if key not in _prog_cache:
        _prog_cache[key] = _build_program(C, D, H)
    nc = _prog_cache[key]

    in_maps = []
    for e in range(E):
        idx = idx_list[e]
        n = len(idx)
        xg = np.zeros((C, D), ml_dtypes.bfloat16)
        xg[:n] = xn[idx].astype(ml_dtypes.bfloat16)
        g = np.zeros(C, np.float32)
        g[:n] = gate_list[e]
        in_maps.append({
            "xgT": np.ascontiguousarray(xg.T),
            "w1": W1[e].astype(ml_dtypes.bfloat16),
            "w2": W2[e].astype(ml_dtypes.bfloat16),
            "b1": b1[e],
            "b2": b2[e],
            "gb": np.ascontiguousarray(np.broadcast_to(g, (P, C))),
        })

    res = bass_utils.run_bass_kernel_spmd(
        nc, in_maps, core_ids=list(range(N_CORES)),
        trace=bool(int(os.environ.get("MOE_TRACE", "0"))))
    if res.exec_time_ns is not None:
        print(f"HW exec time: {res.exec_time_ns} ns")

    # combine: out[t] = x[t] + y[slotA(t)] + y[slotB(t)]
    Y = np.stack([res.results[e]["yT"] for e in range(E)])  # (E, D, C)
    Yf = np.ascontiguousarray(Y.transpose(0, 2, 1)).reshape(E * C, D)
    out = xf + Yf[i1 * C + posA] + Yf[i2 * C + posB]
    return out.reshape(B, L, D).astype(np.float32), bal
